# revision 8
# baseline (speedup 1.0000x reference)
"""Trainium2 Bass kernel for nn_CA_Model (neural cellular automaton).

Strategy: pure data-parallel over batch (8 images -> 8 cores). State is bf16.
The 3x3 perceive + first MLP layer fuse into TWO PSUM-accumulated matmuls per
output row pair via a dual-shift layout X4:

  X4[p = dlt*64 + gg*16 + c, f = tt*258 + w] = xpad[4*tt + gg, w + dlt]

i.e. 4-row blocks on partition groups, with a second column-shifted copy of x
on the upper 64 partitions. A matmul at free offset o reads, for output col j,
tap xpad[row, j+o] on dlt=0 rows and xpad[row, j+o+1] on dlt=1 rows - so one
matmul covers two of the three horizontal taps (K = 2*3rows*16ch = 96 of 128
used) and a second matmul (K=48) covers the third. Rows whose 3-row vertical
tap window straddles a 4-block boundary (row%4 in {3,0}) use X4b, a 2-row
shifted sibling. Layer 2 + the alive/life machinery follow the baseline
scheme (life broadcast via matmul, alive maxpools on a rows-on-partition
alpha tile with DMA partition shifts).
"""
import sys
for _p in ("/opt/trn_rl_repo", "/root/.axon_site/_ro/trn_rl_repo"):
    if _p not in sys.path:
        sys.path.append(_p)

import numpy as np
import ml_dtypes

BF = ml_dtypes.bfloat16
C = 16
HID = 128
H = W = 256
NB = 33            # 8-row blocks in Xc layout (33*8 = 264 row slots)
FW = 258           # padded row width in free dim
FSZ = NB * FW      # 8514 free elements per partition (Xc)
NB4 = 65           # 4-row blocks in X4 layout (65*4 = 260 row slots)
FSZ4 = NB4 * FW    # 16770


def _sobel():
    dx = np.outer([1, 2, 1], [-1, 0, 1]) / 8.0
    f1 = dx.T.astype(np.float32)   # angle=0: F1 = dx.T
    f2 = dx.astype(np.float32)     # F2 = dx
    return f1, f2


def build_weights(W0, b0, W1):
    """Host-side preprocessing of the MLP weights into lhsT tensors."""
    F1, F2 = _sobel()
    W0x, W0y1, W0y2 = W0[:, 0:16], W0[:, 16:32], W0[:, 32:48]
    # A[di][dj]: [HID, C] applied to xpad[row-1+di, wcol-1+dj]
    A = [[(np.float32(di == 1 and dj == 1) * W0x
           + F1[di, dj] * W0y1 + F2[di, dj] * W0y2).astype(np.float32)
          for dj in range(3)] for di in range(3)]

    # L1 lhsT variants: matmul "A" covers dj in {0,1} via the two dlt slots,
    # matmul "B" covers dj=2 via the dlt=1 slot at free offset +1.
    # se in {1,2}: the row's slot within its 4-block (taps gg = se-1+di).
    l1 = []
    for se in (1, 2):
        L = np.zeros((128, 128), np.float32)
        for dlt in range(2):
            for di in range(3):
                r0 = dlt * 64 + 16 * (se - 1 + di)
                L[r0:r0 + 16, :] = A[di][dlt].T
        l1.append(L)
    for se in (1, 2):
        L = np.zeros((128, 128), np.float32)
        for di in range(3):
            r0 = 64 + 16 * (se - 1 + di)
            L[r0:r0 + 16, :] = A[di][2].T
        l1.append(L)

    # layer-2 lhsT: for a row with group g, W1pad[g][n, 16g+c] = W1[c, n]
    w1p = []
    for g in range(8):
        Wp = np.zeros((128, 128), np.float32)
        Wp[:, 16 * g:16 * g + 16] = W1.T
        w1p.append(Wp)

    # life-broadcast lhsT variants (same as baseline): for block tb, rows
    # rho=8tb+g (real rows only), life value lives at LifeQ[q = rho-1,
    # half = q//128] with q2-partition layout (q%8)*16 + (q//8)%16.
    life_plan = []   # per tb: list of (half, lhsT_index)
    r_mats = []
    for tb in range(NB):
        plan = []
        buckets = {}
        for g in range(8):
            rho = 8 * tb + g
            if rho < 1 or rho > 256:
                continue
            q = rho - 1
            half, qh = q // 128, q % 128
            buckets.setdefault(half, []).append((g, qh))
        for half, rows in sorted(buckets.items()):
            Rm = np.zeros((128, 128), np.float32)
            for g, qh in rows:
                q2 = (qh % 8) * 16 + qh // 8
                Rm[q2, 16 * g:16 * g + 16] = 1.0
            plan.append((half, len(r_mats)))
            r_mats.append(Rm)
        life_plan.append(plan)

    l1_stack = np.stack(l1).astype(BF)                 # [4, 128, 128]
    w1_stack = np.stack(w1p).astype(BF)                # [8, 128, 128]
    r_stack = np.stack(r_mats).astype(BF)              # [NR, 128, 128]
    return dict(l1_stack=l1_stack, w1_stack=w1_stack,
                r_stack=r_stack, life_plan=life_plan,
                b0=b0.reshape(128, 1).astype(np.float32))


def marshal_x(img):
    """[256,256,16] image -> Xc [128, FSZ] bf16 interleaved layout."""
    xp = np.zeros((NB * 8, FW, C), np.float32)
    xp[1:257, 1:257, :] = img
    xc = xp.reshape(NB, 8, FW, C).transpose(1, 3, 0, 2).reshape(128, FSZ)
    return np.ascontiguousarray(xc.astype(BF))


def marshal_x4(img, shift):
    """[256,256,16] image -> X4/X4b [128, FSZ4] bf16 dual-shift layout.

    shift=0 -> X4 (row slot = 4*tt+gg), shift=2 -> X4b (row 4*tt+gg-2).
    """
    xp = np.zeros((NB4 * 4 + 2, FW + 1, C), np.float32)
    xp[1 + shift:257 + shift, 1:257, :] = img
    rows = xp[:NB4 * 4]                                    # [260, 259, C]
    x4 = np.zeros((2, 4, C, NB4, FW), np.float32)
    r4 = rows.reshape(NB4, 4, FW + 1, C)
    for dlt in range(2):
        x4[dlt] = r4[:, :, dlt:dlt + FW, :].transpose(1, 3, 0, 2)
    return np.ascontiguousarray(x4.reshape(128, FSZ4).astype(BF))


def unmarshal_x(xc):
    """Xc [128, FSZ] f32 -> [256,256,16] image."""
    xp = xc.reshape(8, C, NB, FW).transpose(2, 0, 3, 1)
    xp = xp.reshape(NB * 8, FW, C)
    return np.ascontiguousarray(xp[1:257, 1:257, :])


def build_program(steps, life_plan, n_r, relu_act_ratio=9,
                  dma_eng1='sync', dma_eng2='scalar'):
    """Returns a compiled Bacc ready for run_bass_kernel_spmd.

    relu_act_ratio: of every 16 relu tiles, this many go to ScalarE (rest DVE).
    """
    import concourse.bass as bass
    import concourse.bacc as bacc
    import concourse.tile as tile
    from concourse import mybir
    F32 = mybir.dt.float32
    BF16 = mybir.dt.bfloat16
    AF = mybir.ActivationFunctionType
    ALU = mybir.AluOpType
    _ = None
    nc = bacc.Bacc(None, target_bir_lowering=False, debug=False, num_devices=8,
                   num_swdge_queues=4)

    xc_ext = nc.declare_dram_parameter("xc", [128, FSZ], BF16, isOutput=False)
    x4_ext = nc.declare_dram_parameter("x4c", [128, FSZ4], BF16, isOutput=False)
    x4b_ext = nc.declare_dram_parameter("x4bc", [128, FSZ4], BF16, isOutput=False)
    l1_ext = nc.declare_dram_parameter("l1w", [4, 128, 128], BF16, isOutput=False)
    w1_ext = nc.declare_dram_parameter("w1w", [8, 128, 128], BF16, isOutput=False)
    r_ext = nc.declare_dram_parameter("rw", [n_r, 128, 128], BF16, isOutput=False)
    b0_ext = nc.declare_dram_parameter("b0w", [128, 1], F32, isOutput=False)
    out_ext = nc.declare_dram_parameter("out", [128, FSZ], F32, isOutput=True)

    de1 = getattr(nc, dma_eng1)
    de2 = getattr(nc, dma_eng2)
    with tile.TileContext(nc) as tc:
        with tc.tile_pool(name="hpool", bufs=3) as hpool, \
             tc.tile_pool(name="lspool", bufs=3) as lspool, \
             tc.tile_pool(name="ph_pool", bufs=3, space="PSUM") as ph_pool, \
             tc.tile_pool(name="pd_pool", bufs=3, space="PSUM") as pd_pool, \
             tc.tile_pool(name="pl_pool", bufs=2, space="PSUM") as pl_pool:

            # --- persistent state (static SBUF allocations) ---
            Xm = nc.alloc_sbuf_tensor("Xm", [128, FSZ], BF16).ap()
            XN = nc.alloc_sbuf_tensor("XN", [128, FSZ], BF16).ap()
            X4 = nc.alloc_sbuf_tensor("X4", [128, FSZ4], BF16).ap()
            X4b = nc.alloc_sbuf_tensor("X4b", [128, FSZ4], BF16).ap()
            OUTF = nc.alloc_sbuf_tensor("OUTF", [128, FSZ], F32).ap()

            LW1 = nc.alloc_sbuf_tensor("LW1", [128, 4 * 128], BF16).ap()
            LW2 = nc.alloc_sbuf_tensor("LW2", [128, 8 * 128], BF16).ap()
            LWR = nc.alloc_sbuf_tensor("LWR", [128, n_r * 128], BF16).ap()
            l1t = [LW1[:, 128 * i:128 * i + 128] for i in range(4)]
            w1t = [LW2[:, 128 * g:128 * g + 128] for g in range(8)]
            rt = [LWR[:, 128 * i:128 * i + 128] for i in range(n_r)]
            b0t = nc.alloc_sbuf_tensor("b0t", [128, 1], F32).ap()

            A_pre = nc.alloc_sbuf_tensor("A_pre", [128, 516], BF16).ap()
            A_post = nc.alloc_sbuf_tensor("A_post", [128, 516], BF16).ap()
            HM = nc.alloc_sbuf_tensor("HM", [128, 512], BF16).ap()
            HMu = nc.alloc_sbuf_tensor("HMu", [128, 512], BF16).ap()
            HMd = nc.alloc_sbuf_tensor("HMd", [128, 512], BF16).ap()
            HMp = nc.alloc_sbuf_tensor("HMp", [128, 512], BF16).ap()
            HMpu = nc.alloc_sbuf_tensor("HMpu", [128, 512], BF16).ap()
            HMpd = nc.alloc_sbuf_tensor("HMpd", [128, 512], BF16).ap()
            # seam scratch: partition 0 holds alpha/hm of rows 128,129
            SEAM = nc.alloc_sbuf_tensor("SEAM", [128, 1032], BF16).ap()
            VMpre = nc.alloc_sbuf_tensor("VMpre", [128, 512], BF16).ap()
            VMpost = nc.alloc_sbuf_tensor("VMpost", [128, 512], BF16).ap()
            LifeQ = nc.alloc_sbuf_tensor("LifeQ", [128, 512], BF16).ap()
            Zrow = nc.alloc_sbuf_tensor("Zrow", [128, 516], BF16).ap()

            # --- loads / init ---
            # Chunked loads so step-0 L1 can chase the X4 stream.
            for lo, n in ((0, 8), (8, 8), (16, 16), (32, 33)):
                de1.dma_start(
                    out=X4[:, lo * FW:(lo + n) * FW],
                    in_=x4_ext[:, lo * FW:(lo + n) * FW])
                de1.dma_start(
                    out=X4b[:, lo * FW:(lo + n) * FW],
                    in_=x4b_ext[:, lo * FW:(lo + n) * FW])
            de2.dma_start(out=Xm[:], in_=xc_ext[:])
            de1.dma_start(out=LW1[:], in_=bass.AP(
                tensor=l1_ext, offset=0,
                ap=[[128, 128], [128 * 128, 4], [1, 128]]))
            de1.dma_start(out=LW2[:], in_=bass.AP(
                tensor=w1_ext, offset=0,
                ap=[[128, 128], [128 * 128, 8], [1, 128]]))
            de1.dma_start(out=LWR[:], in_=bass.AP(
                tensor=r_ext, offset=0,
                ap=[[128, 128], [128 * 128, n_r], [1, 128]]))
            de1.dma_start(out=b0t[:], in_=b0_ext[:])
            nc.vector.memset(Zrow[:], 0.0)
            nc.vector.memset(SEAM[0:32, :], 0.0)
            nc.vector.memset(A_post[:], 0.0)
            nc.vector.memset(A_pre[:], 0.0)

            relu_ctr = [0]

            def relu_tile(dst, src):
                use_act = (relu_ctr[0] % 16) < relu_act_ratio
                relu_ctr[0] += 1
                if use_act:
                    nc.scalar.activation(dst, src, AF.Relu, bias=b0t[:], scale=1.0)
                else:
                    nc.vector.tensor_scalar(dst, src, b0t[:], 0.0,
                                            op0=ALU.add, op1=ALU.max)

            def extract_alpha(dst_A, src_X, halves=(0, 1)):
                # q2-layout: dst_A[q2 = gp*16 + j, half*258 + 1 + w] holds
                # alpha of row rho = 128*half + 8j + gp + 1.
                for half in halves:
                    for gp in range(8):
                        g = (gp + 1) % 8
                        t0 = 16 * half + (1 if gp == 7 else 0)
                        dst = bass.AP(
                            tensor=dst_A.tensor,
                            offset=16 * gp * 516 + 258 * half + 1,
                            ap=[[516, 16], [1, 256]])
                        src = bass.AP(
                            tensor=src_X.tensor,
                            offset=(16 * g + 3) * FSZ + t0 * FW + 1,
                            ap=[[FSZ, 1], [FW, 16], [1, 256]])
                        de1.dma_start(out=dst, in_=src)

            def pool_half(dst_VM, src_A, half, hm, hmu, hmd):
                """maxpool one half; seam values come from the SEAM tile."""
                lo = 258 * half
                qlo, qhi = 256 * half, 256 * half + 256
                av = src_A[:, lo:lo + 258]
                nc.vector.tensor_tensor(hm[:, qlo:qhi], av[:, 0:256],
                                        av[:, 2:258], op=ALU.max)
                nc.vector.tensor_tensor(hm[:, qlo:qhi], hm[:, qlo:qhi],
                                        av[:, 1:257], op=ALU.max)
                nc.gpsimd.dma_start(out=hmu[0:112, qlo:qhi], in_=hm[16:128, qlo:qhi])
                nc.gpsimd.dma_start(out=hmu[112:127, qlo:qhi], in_=hm[1:16, qlo:qhi])
                nc.gpsimd.dma_start(out=hmd[16:128, qlo:qhi], in_=hm[0:112, qlo:qhi])
                nc.gpsimd.dma_start(out=hmd[1:16, qlo:qhi], in_=hm[112:127, qlo:qhi])
                if half == 0:
                    nc.gpsimd.dma_start(out=hmu[127:128, 0:256],
                                        in_=SEAM[0:1, 775:1031])
                    nc.gpsimd.dma_start(out=hmd[0:1, 0:256], in_=Zrow[0:1, 0:256])
                else:
                    nc.gpsimd.dma_start(out=hmd[0:1, 256:512],
                                        in_=SEAM[0:1, 517:773])
                    nc.gpsimd.dma_start(out=hmu[127:128, 256:512],
                                        in_=Zrow[0:1, 0:256])
                nc.vector.tensor_tensor(dst_VM[:, qlo:qhi], hm[:, qlo:qhi],
                                        hmu[:, qlo:qhi], op=ALU.max)
                nc.vector.tensor_tensor(dst_VM[:, qlo:qhi], dst_VM[:, qlo:qhi],
                                        hmd[:, qlo:qhi], op=ALU.max)

            def seam_fill_from_A(src_A):
                # rows 128 (q2=127, h0) / 129 (q2=0, h1) from the A tile
                nc.gpsimd.dma_start(out=SEAM[0:1, 1:257],
                                    in_=src_A[127:128, 1:257])
                nc.gpsimd.dma_start(out=SEAM[0:1, 259:515],
                                    in_=src_A[0:1, 259:515])
                sv = SEAM[0:1, :].rearrange("p (a w) -> p a w", a=4)
                nc.vector.tensor_tensor(sv[:, 2:4, 1:257], sv[:, 0:2, 0:256],
                                        sv[:, 0:2, 2:258], op=ALU.max)
                nc.vector.tensor_tensor(sv[:, 2:4, 1:257], sv[:, 2:4, 1:257],
                                        sv[:, 0:2, 1:257], op=ALU.max)

            def seam_fill(src_X):
                # alpha rows 128 (g 0, t 16) and 129 (g 1, t 16) -> SEAM p0
                nc.gpsimd.dma_start(
                    out=SEAM[0:1, 1:257],
                    in_=bass.AP(tensor=src_X.tensor,
                                offset=3 * FSZ + 16 * FW + 1,
                                ap=[[FSZ, 1], [1, 256]]))
                nc.gpsimd.dma_start(
                    out=SEAM[0:1, 259:515],
                    in_=bass.AP(tensor=src_X.tensor,
                                offset=(16 + 3) * FSZ + 16 * FW + 1,
                                ap=[[FSZ, 1], [1, 256]]))
                sv = SEAM[0:1, :].rearrange("p (a w) -> p a w", a=4)
                nc.vector.tensor_tensor(sv[:, 2:4, 1:257], sv[:, 0:2, 0:256],
                                        sv[:, 0:2, 2:258], op=ALU.max)
                nc.vector.tensor_tensor(sv[:, 2:4, 1:257], sv[:, 2:4, 1:257],
                                        sv[:, 0:2, 1:257], op=ALU.max)

            def x4_update(tb):
                """Refresh X4/X4b blocks 2tb, 2tb+1 from Xm block tb."""
                # X4 blocks 2tb (rows 0..3) / 2tb+1 (rows 4..7) <- Xm block tb;
                # the dlt=1 copy reads one src column to the right.
                for half in range(2 if tb < NB - 1 else 1):
                    for dlt in range(2):
                        dst = bass.AP(tensor=X4.tensor,
                                      offset=dlt * 64 * FSZ4 + (2 * tb + half) * FW,
                                      ap=[[FSZ4, 64], [1, 257]])
                        src = bass.AP(tensor=Xm.tensor,
                                      offset=half * 64 * FSZ + tb * FW + dlt,
                                      ap=[[FSZ, 64], [1, 257]])
                        de1.dma_start(out=dst, in_=src)
                # X4b refresh, per dlt half (single partition-dim APs only)
                nblk = 1 if tb == NB - 1 else 2
                for dlt in range(2):
                    base = dlt * 64 * FSZ4
                    dst = bass.AP(tensor=X4b.tensor,
                                  offset=base + 32 * FSZ4 + 2 * tb * FW,
                                  ap=[[FSZ4, 32], [FW, nblk], [1, 257]])
                    src = bass.AP(tensor=X4.tensor, offset=base + 2 * tb * FW,
                                  ap=[[FSZ4, 32], [FW, nblk], [1, 257]])
                    de2.dma_start(out=dst, in_=src)
                    if tb < NB - 1:
                        dst = bass.AP(tensor=X4b.tensor,
                                      offset=base + (2 * tb + 1) * FW,
                                      ap=[[FSZ4, 32], [FW, 2], [1, 257]])
                        src = bass.AP(tensor=X4.tensor,
                                      offset=base + 32 * FSZ4 + 2 * tb * FW,
                                      ap=[[FSZ4, 32], [FW, 2], [1, 257]])
                        de2.dma_start(out=dst, in_=src)

            for step in range(steps):
                last_step = step + 1 == steps

                # --- pre pool ---
                if step == 0:
                    extract_alpha(A_pre, Xm)
                seam_fill_from_A(A_pre)
                pool_half(VMpre, A_pre, 0, HM, HMu, HMd)
                pool_half(VMpre, A_pre, 1, HM, HMu, HMd)

                # --- main sweep: layer1 + relu + layer2 ---
                d_tiles = {}
                d_count = {}
                d_expect = {tb: 8 for tb in range(NB)}
                d_expect[0] = 7
                d_expect[32] = 1

                def life_block(tb):
                    lo = tb * FW + 1
                    plan = life_plan[tb]
                    pl = pl_pool.tile([128, 256], F32,
                                      name=f"pl_{step}_{tb}", tag="pl")
                    for i, (half, ridx) in enumerate(plan):
                        nc.tensor.matmul(
                            pl[:], rt[ridx],
                            LifeQ[:, half * 256:half * 256 + 256],
                            start=(i == 0), stop=(i == len(plan) - 1))
                    ls = lspool.tile([128, 256], BF16,
                                     name=f"ls_{step}_{tb}", tag="ls")
                    nc.scalar.copy(ls[:], pl[:])
                    if last_step:
                        nc.vector.tensor_tensor(OUTF[:, lo:lo + 256],
                                                XN[:, lo:lo + 256], ls[:],
                                                op=ALU.mult)
                        de1.dma_start(out=out_ext[:, lo:lo + 256],
                                          in_=OUTF[:, lo:lo + 256])
                    else:
                        nc.vector.tensor_tensor(Xm[:, lo:lo + 256],
                                                XN[:, lo:lo + 256], ls[:],
                                                op=ALU.mult)
                        x4_update(tb)

                def post_half(half):
                    extract_alpha(A_post, XN, halves=(half,))
                    pool_half(VMpost, A_post, half, HMp, HMpu, HMpd)
                    qlo = 256 * half
                    qs = slice(qlo, qlo + 256)
                    nc.vector.tensor_tensor(LifeQ[:, qs], VMpre[:, qs],
                                            VMpost[:, qs], op=ALU.min)
                    nc.vector.tensor_scalar(LifeQ[:, qs], LifeQ[:, qs],
                                            0.1, None, op0=ALU.is_gt)
                    if not last_step:
                        flo = 258 * half + 1
                        nc.vector.tensor_tensor(A_pre[:, flo:flo + 256],
                                                A_post[:, flo:flo + 256],
                                                LifeQ[:, qs], op=ALU.mult)
                    if half == 0:
                        for tb in range(0, 16):
                            life_block(tb)
                    else:
                        for tb in list(range(17, NB)) + [16]:
                            life_block(tb)

                def l2(rho, ht, hslice):
                    tb, g = rho // 8, rho % 8
                    if tb not in d_tiles:
                        d_tiles[tb] = pd_pool.tile([128, 256], F32,
                                                   name=f"pd_s{step}_{tb}",
                                                   tag="pd")
                        d_count[tb] = 0
                    first = d_count[tb] == 0
                    d_count[tb] += 1
                    last = d_count[tb] == d_expect[tb]
                    nc.tensor.matmul(d_tiles[tb][:], w1t[g][:],
                                     ht[:, hslice], start=first, stop=last)
                    if last:
                        lo = tb * FW + 1
                        nc.vector.tensor_tensor(
                            XN[:, lo:lo + 256], d_tiles[tb][:],
                            Xm[:, lo:lo + 256], op=ALU.add)
                        if tb == 16:
                            seam_fill(XN)
                            post_half(0)

                # L1 lhsT index: [A_se1, A_se2, B_se1, B_se2]
                for tp in range(16):
                    for s in range(8):
                        rho = 16 * tp + s + 1
                        se = rho % 4
                        if se in (1, 2):
                            src, sew = X4, se
                            t0 = (rho - se) // 4
                        else:
                            sew = (se + 2) % 4           # 1 or 2
                            src, t0 = X4b, (rho + 2 - sew) // 4
                        ph = ph_pool.tile([128, 2, 256], F32, tag="ph")
                        rhs_a = bass.AP(tensor=src.tensor, offset=t0 * FW,
                                        ap=[[FSZ4, 128], [2 * FW, 2], [1, 256]])
                        rhs_b = bass.AP(tensor=src.tensor, offset=t0 * FW + 1,
                                        ap=[[FSZ4, 128], [2 * FW, 2], [1, 256]])
                        nc.tensor.matmul(ph[:], l1t[sew - 1][:], rhs_a,
                                         start=True, stop=False)
                        nc.tensor.matmul(ph[:], l1t[2 + sew - 1][:], rhs_b,
                                         start=False, stop=True)
                        ht = hpool.tile([128, 512], BF16, tag="ht")
                        relu_tile(ht[:], ph.rearrange("p a b -> p (a b)"))
                        l2(rho, ht, slice(0, 256))
                        l2(rho + 8, ht, slice(256, 512))

                # h0's post chain was emitted mid-sweep (block 16); finish h1
                post_half(1)

    nc.compile()
    return nc


_PROGRAM_CACHE = {}


def kernel(x, W0, b0, W1, steps, _trace=False):
    import concourse.bass_utils as bass_utils
    steps = int(steps)
    x = np.asarray(x, dtype=np.float32)
    W0 = np.asarray(W0, dtype=np.float32)
    b0 = np.asarray(b0, dtype=np.float32)
    W1 = np.asarray(W1, dtype=np.float32)
    B = x.shape[0]
    assert x.shape == (8, H, W, C), x.shape
    assert steps >= 1

    wts = build_weights(W0, b0, W1)
    key = steps
    if key not in _PROGRAM_CACHE:
        _PROGRAM_CACHE[key] = build_program(steps, wts["life_plan"],
                                            wts["r_stack"].shape[0])
    nc = _PROGRAM_CACHE[key]

    in_maps = []
    for b in range(B):
        in_maps.append({
            "xc": marshal_x(x[b]),
            "x4c": marshal_x4(x[b], 0),
            "x4bc": marshal_x4(x[b], 2),
            "l1w": wts["l1_stack"],
            "w1w": wts["w1_stack"],
            "rw": wts["r_stack"],
            "b0w": wts["b0"],
        })
    res = bass_utils.run_bass_kernel_spmd(nc, in_maps, list(range(8)),
                                          trace=_trace)
    kernel.last_result = res
    out = np.stack([unmarshal_x(res.results[b]["out"]) for b in range(B)])
    return out.astype(np.float32)
